# revision 1
# baseline (speedup 1.0000x reference)
"""YOLO-v2 loss kernel for Trainium2 (8 NeuronCores, data-parallel over batch).

Layout insight: pyolos [B, 425, 26, 26] is [B, ch*5anc, hw] with plane = c*5+a.
The loss needs:
  - conf channel (planes 0..4) densely: sum of sigmoid(conf)^2 over all
    positions (background term; gconf == 0 exactly wherever no GT matched).
  - cls/txywh channels only at the <=8 matched (cell, anchor) slots per image.
So each core reads 16 conf-plane blocks (216KB) + an indirect gather of
128 slots x 85 channels instead of the full 18.4MB chunk.

Per-core partial sums (8 f32) are combined on the host (the all-reduce-mean
step of the data-parallel recipe).
"""

import numpy as np

from concourse import bass, mybir
from concourse.bass_utils import run_bass_kernel_spmd
from concourse.masks import make_identity
from concourse.tile import TileContext

F32 = mybir.dt.float32
I32 = mybir.dt.int32
AF = mybir.ActivationFunctionType
OP = mybir.AluOpType
AX = mybir.AxisListType

NC = 8                 # cores
B = 128                # batch
BL = B // NC           # images per core (16)
NGT = 8                # GTs per image
S = BL * NGT           # slots per core (128)
GRID = 26
HW = GRID * GRID       # 676
NCH = 85               # conf + 80 cls + 4 txywh
NANC = 5
IMG = 425 * HW         # elements per image (287300)
EPS = 1e-7
ANC = np.array([[0.05, 0.07], [0.12, 0.15], [0.25, 0.30],
                [0.45, 0.50], [0.80, 0.85]], np.float32)

# ---- consts tensor column layout ----
C_UT = 0            # [128] strict-upper-triangular within image blocks
C_IOTA5 = 128       # [5]
C_AW = 133          # [5]
C_AH = 138          # [5]
C_AREA = 143        # [5]
C_IOTA80 = 148      # [80]
C_POW2 = 228        # [5]
C_POW2I = 233       # [5]
C_CHOFF = 238       # [85] per-slot channel offsets (incl. image base)
C_ONES = 323        # [1]
C_IOTA26 = 324      # [26]
C_HALF = 350        # [1] = 0.5
C_GB = 351          # [4] per-slot gbox ltrb
C_LBL = 355         # [1] per-slot label
C_IOTAM99 = 356     # [5] iota5 - 99
NCONST = 361
# gathered channel order: [conf, tx, ty, tw, th, cls0..cls79]
CH_ORDER = [0, 81, 82, 83, 84] + list(range(1, 81))


def _make_consts(gbx_core: np.ndarray, lbl_core: np.ndarray) -> np.ndarray:
    ct = np.zeros((S, NCONST), np.float32)
    ct[:, C_GB:C_GB + 4] = gbx_core
    ct[:, C_LBL] = lbl_core
    i = np.arange(S)
    j = np.arange(S)
    ct[:, C_UT:C_UT + S] = ((i[:, None] // NGT == j[None, :] // NGT)
                            & (j[None, :] > i[:, None])).astype(np.float32)
    ct[:, C_IOTA5:C_IOTA5 + 5] = np.arange(5, dtype=np.float32)[None, :]
    ct[:, C_AW:C_AW + 5] = ANC[:, 0][None, :]
    ct[:, C_AH:C_AH + 5] = ANC[:, 1][None, :]
    ct[:, C_AREA:C_AREA + 5] = (ANC[:, 0] * ANC[:, 1])[None, :]
    ct[:, C_IOTA80:C_IOTA80 + 80] = np.arange(80, dtype=np.float32)[None, :]
    ct[:, C_POW2:C_POW2 + 5] = (2.0 ** np.arange(5))[None, :]
    ct[:, C_POW2I:C_POW2I + 5] = (0.5 ** np.arange(5))[None, :]
    ct[:, C_CHOFF:C_CHOFF + NCH] = ((i // NGT)[:, None] * IMG
                                    + np.array(CH_ORDER)[None, :] * (5 * HW))
    ct[:, C_ONES] = 1.0
    ct[:, C_IOTA26:C_IOTA26 + GRID] = np.arange(GRID, dtype=np.float32)[None]
    ct[:, C_HALF] = 0.5
    ct[:, C_IOTAM99:C_IOTAM99 + 5] = np.arange(5, dtype=np.float32)[None] - 99.0
    return ct


def _split_multiwaits(nc: bass.Bass, k: int = 1) -> None:
    """This walrus build rejects instructions with >~2 sync waits; hoist
    extra waits onto preceding same-engine NoOps (equivalent for monotone
    sem-ge waits)."""
    for fn in nc.m.functions:
        for bb in fn.blocks:
            out = []
            for inst in bb.instructions:
                si = inst.sync_info
                waits = list(si.on_wait) if si is not None and si.on_wait else []
                if len(waits) > k:
                    for i, w in enumerate(waits[:-k]):
                        out.append(mybir.InstNoOp(
                            name=f"{inst.name}-wsplit{i}",
                            engine=inst.engine,
                            bass_nofuse=True,
                            sync_info=mybir.SyncInfo(on_wait=[w],
                                                     on_update=[]),
                        ))
                    inst.sync_info = mybir.SyncInfo(
                        on_wait=waits[-k:], on_update=list(si.on_update))
                out.append(inst)
            bb.instructions = out


def build_bass() -> bass.Bass:
    nc = bass.Bass()
    py = nc.declare_dram_parameter("pyolos", [BL, 425, HW], F32, isOutput=False)
    cn = nc.declare_dram_parameter("consts", [S, NCONST], F32, isOutput=False)
    out = nc.declare_dram_parameter("out", [1, 8], F32, isOutput=True)
    py_flat = py[:, :, :].rearrange("a b c -> (a b c)")

    with TileContext(nc) as tc:
        with (
            tc.tile_pool(name="sb", bufs=1) as sb,
            tc.tile_pool(name="ps", bufs=1, space="PSUM") as ps,
        ):
            ct = sb.tile([S, NCONST], F32, name="ct")
            nc.sync.dma_start(out=ct[:], in_=cn[:, :])
            ident = sb.tile([S, S], F32, name="ident")
            make_identity(nc, ident[:])

            # ---------------- dense conf term ----------------
            conf = sb.tile([BL * 5, HW], F32, name="conf")
            nc.sync.dma_start(out=conf[:], in_=py[:, 0:5, :])
            # sigmoid(x)^2 = exp(-2*softplus(-x)); Exp/Ln only so every ACT
            # op in the kernel shares one table set (no per-op table reloads)
            sigc = sb.tile([BL * 5, HW], F32, name="sigc")
            nc.scalar.activation(sigc[:], conf[:], AF.Exp, scale=-1.0)
            nc.scalar.activation(sigc[:], sigc[:], AF.Ln, bias=1.0)
            sq80 = sb.tile([BL * 5, HW], F32, name="sq80")
            densesq = sb.tile([BL * 5, 1], F32, name="densesq")
            nc.scalar.activation(sq80[:], sigc[:], AF.Exp, scale=-2.0,
                                 accum_out=densesq[:])

            # ---------------- matching (slot layout [128, *]) ----------------
            def tt(shape, tag):
                return sb.tile(shape, F32, name=tag)

            c26 = tt([S, 2], "c26")
            nc.vector.tensor_tensor(out=c26[:], in0=ct[:, C_GB:C_GB + 2],
                                    in1=ct[:, C_GB + 2:C_GB + 4], op=OP.add)
            nc.vector.tensor_scalar_mul(c26[:], c26[:], 13.0)
            wh = tt([S, 2], "wh")
            nc.vector.tensor_tensor(out=wh[:], in0=ct[:, C_GB + 2:C_GB + 4],
                                    in1=ct[:, C_GB:C_GB + 2], op=OP.subtract)

            # floor(c26) via compare-count: sum_k [iota26 <= x] - 1
            colrow = tt([S, 2], "colrow")
            ge26 = tt([S, GRID], "ge26")
            for d in range(2):
                nc.vector.tensor_scalar(ge26[:], ct[:, C_IOTA26:C_IOTA26 + GRID],
                                        c26[:, d:d + 1], None, OP.is_le)
                nc.vector.tensor_reduce(colrow[:, d:d + 1], ge26[:], AX.X,
                                        OP.add)
            nc.vector.tensor_scalar(colrow[:], colrow[:], -1.0, None, OP.add)
            txy = tt([S, 2], "txy")
            nc.vector.tensor_tensor(out=txy[:], in0=c26[:], in1=colrow[:],
                                    op=OP.subtract)
            cell = tt([S, 1], "cell")
            nc.vector.tensor_scalar_mul(cell[:], colrow[:, 1:2], float(GRID))
            nc.vector.tensor_tensor(out=cell[:], in0=cell[:],
                                    in1=colrow[:, 0:1], op=OP.add)

            inw = tt([S, 5], "inw")
            nc.vector.tensor_scalar(inw[:], ct[:, C_AW:C_AW + 5],
                                    wh[:, 0:1], None, OP.min)
            inh = tt([S, 5], "inh")
            nc.vector.tensor_scalar(inh[:], ct[:, C_AH:C_AH + 5],
                                    wh[:, 1:2], None, OP.min)
            inter = tt([S, 5], "inter")
            nc.vector.tensor_tensor(out=inter[:], in0=inw[:], in1=inh[:],
                                    op=OP.mult)
            areag = tt([S, 1], "areag")
            nc.vector.tensor_tensor(out=areag[:], in0=wh[:, 0:1],
                                    in1=wh[:, 1:2], op=OP.mult)
            den = tt([S, 5], "den")
            nc.vector.tensor_tensor(out=den[:], in0=ct[:, C_AREA:C_AREA + 5],
                                    in1=inter[:], op=OP.subtract)
            nc.vector.tensor_scalar(den[:], den[:], areag[:, 0:1], EPS,
                                    OP.add, OP.add)
            deni = tt([S, 5], "deni")
            nc.vector.reciprocal(deni[:], den[:])
            iou2 = tt([S, 5], "iou2")
            nc.vector.tensor_tensor(out=iou2[:], in0=inter[:], in1=deni[:],
                                    op=OP.mult)
            mign = tt([S, 5], "mign")
            nc.vector.tensor_scalar(mign[:], iou2[:], 0.5, None, OP.is_gt)
            mx = tt([S, 1], "mx")
            nc.vector.tensor_reduce(mx[:], iou2[:], AX.X, OP.max)
            eqm = tt([S, 5], "eqm")
            nc.vector.tensor_scalar(eqm[:], iou2[:], mx[:, 0:1], None,
                                    OP.is_equal)
            # first argmax: min over (iota if eq else 99)
            tsel = tt([S, 5], "tsel")
            nc.vector.tensor_tensor(out=tsel[:], in0=ct[:, C_IOTAM99:C_IOTAM99 + 5],
                                    in1=eqm[:], op=OP.mult)
            nc.vector.tensor_scalar(tsel[:], tsel[:], 99.0, None, OP.add)
            idxm = tt([S, 1], "idxm")
            nc.vector.tensor_reduce(idxm[:], tsel[:], AX.X, OP.min)
            acell = tt([S, 1], "acell")
            nc.vector.tensor_scalar_mul(acell[:], idxm[:], float(HW))
            nc.vector.tensor_tensor(out=acell[:], in0=acell[:], in1=cell[:],
                                    op=OP.add)
            offs_f = tt([S, NCH], "offs_f")
            nc.vector.tensor_scalar(offs_f[:], ct[:, C_CHOFF:C_CHOFF + NCH],
                                    acell[:, 0:1], None, OP.add)
            offs_i = sb.tile([S, NCH], I32, name="offs_i")
            nc.vector.tensor_copy(offs_i[:], offs_f[:])
            pf = sb.tile([S, NCH], F32, name="pf")
            nc.gpsimd.indirect_dma_start(
                out=pf[:], out_offset=None,
                in_=py_flat.rearrange("(a b) -> a b", b=1),
                in_offset=bass.IndirectOffsetOnAxis(ap=offs_i[:, :],
                                                    axis=0))

            # ---------------- per-slot loss terms ----------------
            # gathered order: [conf, tx, ty, tw, th, cls0..79]
            onehot5 = tt([S, 5], "onehot5")
            nc.vector.tensor_scalar(onehot5[:], ct[:, C_IOTA5:C_IOTA5 + 5],
                                    idxm[:, 0:1], None, OP.is_equal)

            ancsel = tt([S, 2], "ancsel")
            scr5 = tt([S, 5], "scr5")
            nc.vector.tensor_tensor(out=scr5[:], in0=onehot5[:],
                                    in1=ct[:, C_AW:C_AW + 5], op=OP.mult)
            nc.vector.tensor_reduce(ancsel[:, 0:1], scr5[:], AX.X, OP.add)
            nc.vector.tensor_tensor(out=scr5[:], in0=onehot5[:],
                                    in1=ct[:, C_AH:C_AH + 5], op=OP.mult)
            nc.vector.tensor_reduce(ancsel[:, 1:2], scr5[:], AX.X, OP.add)
            ancinv = tt([S, 2], "ancinv")
            nc.vector.reciprocal(ancinv[:], ancsel[:])
            twh = tt([S, 2], "twh")
            nc.vector.tensor_tensor(out=twh[:], in0=wh[:], in1=ancinv[:],
                                    op=OP.mult)
            nc.scalar.activation(twh[:], twh[:], AF.Ln)
            weight = tt([S, 1], "weight")
            nc.vector.tensor_scalar(weight[:], areag[:], -1.0, 2.0,
                                    OP.mult, OP.add)

            key = tt([S, 1], "key")
            nc.vector.tensor_scalar_mul(key[:], cell[:], 5.0)
            nc.vector.tensor_tensor(out=key[:], in0=key[:], in1=idxm[:],
                                    op=OP.add)

            # ---------------- cross-slot logic (PE transposes) -------------
            def transpose_col(src, tag):
                p = ps.tile([S, S], F32, name=tag + "_p")
                nc.tensor.transpose(out=p[:],
                                    in_=src[:, 0:1].to_broadcast([S, S]),
                                    identity=ident[:])
                t = sb.tile([S, S], F32, name=tag)
                nc.vector.tensor_copy(t[:], p[:])
                return t

            keyT = transpose_col(key, "keyT")
            cellT = transpose_col(cell, "cellT")

            eqkey = tt([S, S], "eqkey")
            nc.vector.tensor_scalar(eqkey[:], keyT[:], key[:, 0:1], None,
                                    OP.is_equal)
            nc.vector.tensor_tensor(out=eqkey[:], in0=eqkey[:],
                                    in1=ct[:, C_UT:C_UT + S], op=OP.mult)
            ovw = tt([S, 1], "ovw")
            nc.vector.tensor_reduce(ovw[:], eqkey[:], AX.X, OP.max)
            lastw = tt([S, 1], "lastw")
            nc.vector.tensor_scalar(lastw[:], ovw[:], -1.0, 1.0,
                                    OP.mult, OP.add)

            # bit[i, j] = mign[j, anc_i] via PE: onehot5^T (x) mign^T matmul
            oh5T_p = ps.tile([5, S], F32, name="oh5T_p")
            nc.tensor.transpose(out=oh5T_p[:], in_=onehot5[:],
                                identity=ident[:])
            oh5T = sb.tile([5, S], F32, name="oh5T")
            nc.vector.tensor_copy(oh5T[:], oh5T_p[:])
            mignT_p = ps.tile([5, S], F32, name="mignT_p")
            nc.tensor.transpose(out=mignT_p[:], in_=mign[:],
                                identity=ident[:])
            mignT = sb.tile([5, S], F32, name="mignT")
            nc.vector.tensor_copy(mignT[:], mignT_p[:])
            bit_p = ps.tile([S, S], F32, name="bit_p")
            nc.tensor.matmul(out=bit_p[:], lhsT=oh5T[:], rhs=mignT[:],
                             start=True, stop=True)
            bit = tt([S, S], "bit")
            nc.vector.tensor_copy(bit[:], bit_p[:])

            eqc = tt([S, S], "eqc")
            nc.vector.tensor_scalar(eqc[:], cellT[:], cell[:, 0:1], None,
                                    OP.is_equal)
            nc.vector.tensor_tensor(out=eqc[:], in0=eqc[:], in1=bit[:],
                                    op=OP.mult)
            nc.vector.tensor_tensor(out=eqc[:], in0=eqc[:],
                                    in1=ct[:, C_UT:C_UT + S], op=OP.mult)
            ignov = tt([S, 1], "ignov")
            nc.vector.tensor_reduce(ignov[:], eqc[:], AX.X, OP.max)
            # weff = weight*(1-ignov) - ignov
            weff = tt([S, 1], "weff")
            nc.vector.tensor_scalar(weff[:], ignov[:], -1.0, 1.0,
                                    OP.mult, OP.add)
            nc.vector.tensor_tensor(out=weff[:], in0=weff[:], in1=weight[:],
                                    op=OP.mult)
            nc.vector.tensor_tensor(out=weff[:], in0=weff[:], in1=ignov[:],
                                    op=OP.subtract)

            # ---------------- indirect gather of 85 channels ---------------
            u3 = tt([S, 3], "u3")
            nc.scalar.activation(u3[:], pf[:, 0:3], AF.Exp, scale=-1.0)
            sig3 = tt([S, 3], "sig3")
            nc.vector.tensor_scalar(sig3[:], u3[:], 1.0, None, OP.add)
            nc.vector.reciprocal(sig3[:], sig3[:])
            pconf = sig3[:, 0:1]
            sxy = sig3[:, 1:3]
            pxy = tt([S, 2], "pxy")
            nc.vector.tensor_tensor(out=pxy[:], in0=sxy, in1=colrow[:],
                                    op=OP.add)
            nc.vector.tensor_scalar_mul(pxy[:], pxy[:], 1.0 / GRID)
            pwh = tt([S, 2], "pwh")
            nc.scalar.activation(pwh[:], pf[:, 3:5], AF.Exp)
            nc.vector.tensor_tensor(out=pwh[:], in0=pwh[:], in1=ancsel[:],
                                    op=OP.mult)
            pwh2 = tt([S, 2], "pwh2")
            nc.vector.tensor_scalar_mul(pwh2[:], pwh[:], 0.5)
            plt = tt([S, 2], "plt")
            nc.vector.tensor_tensor(out=plt[:], in0=pxy[:], in1=pwh2[:],
                                    op=OP.subtract)
            prb = tt([S, 2], "prb")
            nc.vector.tensor_tensor(out=prb[:], in0=pxy[:], in1=pwh2[:],
                                    op=OP.add)
            ilt = tt([S, 2], "ilt")
            nc.vector.tensor_tensor(out=ilt[:], in0=plt[:],
                                    in1=ct[:, C_GB:C_GB + 2],
                                    op=OP.max)
            irb = tt([S, 2], "irb")
            nc.vector.tensor_tensor(out=irb[:], in0=prb[:],
                                    in1=ct[:, C_GB + 2:C_GB + 4],
                                    op=OP.min)
            iwh = tt([S, 2], "iwh")
            nc.vector.tensor_tensor(out=iwh[:], in0=irb[:], in1=ilt[:],
                                    op=OP.subtract)
            nc.vector.tensor_scalar(iwh[:], iwh[:], 0.0, None, OP.max)
            inter2 = tt([S, 1], "inter2")
            nc.vector.tensor_tensor(out=inter2[:], in0=iwh[:, 0:1],
                                    in1=iwh[:, 1:2], op=OP.mult)
            pa = tt([S, 1], "pa")
            nc.vector.tensor_tensor(out=pa[:], in0=pwh[:, 0:1],
                                    in1=pwh[:, 1:2], op=OP.mult)
            den2 = tt([S, 1], "den2")
            nc.vector.tensor_tensor(out=den2[:], in0=areag[:], in1=inter2[:],
                                    op=OP.subtract)
            nc.vector.tensor_scalar(den2[:], den2[:], pa[:, 0:1], EPS,
                                    OP.add, OP.add)
            den2i = tt([S, 1], "den2i")
            nc.vector.reciprocal(den2i[:], den2[:])
            gconf = tt([S, 1], "gconf")
            nc.vector.tensor_tensor(out=gconf[:], in0=inter2[:], in1=den2i[:],
                                    op=OP.mult)
            gpos = tt([S, 1], "gpos")
            nc.vector.tensor_scalar(gpos[:], gconf[:], 0.0, None, OP.is_gt)
            mp = tt([S, 1], "mp")
            nc.vector.tensor_tensor(out=mp[:], in0=lastw[:], in1=gpos[:],
                                    op=OP.mult)
            mpw = tt([S, 1], "mpw")
            nc.vector.tensor_tensor(out=mpw[:], in0=mp[:], in1=weff[:],
                                    op=OP.mult)

            stack = sb.tile([S, 8], F32, name="stack")
            nc.vector.memset(stack[:], 0.0)
            nc.vector.tensor_copy(stack[0:BL * 5, 0:1], densesq[:])

            dconf = tt([S, 1], "dconf")
            nc.vector.tensor_scalar(dconf[:], pconf, gconf[:, 0:1], None,
                                    OP.subtract)
            nc.vector.tensor_tensor(out=dconf[:], in0=dconf[:], in1=dconf[:],
                                    op=OP.mult)
            nc.vector.tensor_tensor(out=stack[:, 1:2], in0=mp[:],
                                    in1=dconf[:], op=OP.mult)
            psq = tt([S, 1], "psq")
            nc.vector.tensor_tensor(out=psq[:], in0=pconf, in1=pconf,
                                    op=OP.mult)
            nc.vector.tensor_tensor(out=stack[:, 2:3], in0=mp[:], in1=psq[:],
                                    op=OP.mult)
            nc.vector.tensor_copy(stack[:, 3:4], mp[:])

            # cls: sum softplus(x_c) - x_label over channels 5..85
            sp80 = tt([S, 80], "sp80")
            spsum = tt([S, 1], "spsum")
            nc.scalar.activation(sp80[:], pf[:, 5:85], AF.Exp)
            nc.scalar.activation(sp80[:], sp80[:], AF.Ln, bias=1.0,
                                 accum_out=spsum[:])   # softplus
            lblm1 = tt([S, 1], "lblm1")
            nc.vector.tensor_scalar(lblm1[:], ct[:, C_LBL:C_LBL + 1], -1.0,
                                    None, OP.add)
            oh80 = tt([S, 80], "oh80")
            nc.vector.tensor_scalar(oh80[:], ct[:, C_IOTA80:C_IOTA80 + 80],
                                    lblm1[:, 0:1], None, OP.is_equal)
            xlab = tt([S, 1], "xlab")
            scr80 = tt([S, 80], "scr80")
            nc.vector.tensor_tensor(out=scr80[:], in0=oh80[:],
                                    in1=pf[:, 5:85], op=OP.mult)
            nc.vector.tensor_reduce(xlab[:], scr80[:], AX.X, OP.add)
            clsn = tt([S, 1], "clsn")
            nc.vector.tensor_tensor(out=clsn[:], in0=spsum[:], in1=xlab[:],
                                    op=OP.subtract)
            nc.vector.tensor_tensor(out=stack[:, 4:5], in0=mp[:], in1=clsn[:],
                                    op=OP.mult)

            # txy bce: softplus(x) - z*x = x + softplus(-x) - z*x; reuse u3
            sptxy = tt([S, 2], "sptxy")
            nc.scalar.activation(sptxy[:], u3[:, 1:3], AF.Ln, bias=1.0)
            nc.vector.tensor_tensor(out=sptxy[:], in0=sptxy[:],
                                    in1=pf[:, 1:3], op=OP.add)
            zx = tt([S, 2], "zx")
            nc.vector.tensor_tensor(out=zx[:], in0=txy[:], in1=pf[:, 1:3],
                                    op=OP.mult)
            nc.vector.tensor_tensor(out=sptxy[:], in0=sptxy[:], in1=zx[:],
                                    op=OP.subtract)
            bcexy = tt([S, 1], "bcexy")
            nc.vector.tensor_reduce(bcexy[:], sptxy[:], AX.X, OP.add)
            nc.vector.tensor_tensor(out=stack[:, 5:6], in0=mpw[:],
                                    in1=bcexy[:], op=OP.mult)

            # twh mse on channels 3:5
            dwh = tt([S, 2], "dwh")
            nc.vector.tensor_tensor(out=dwh[:], in0=pf[:, 3:5], in1=twh[:],
                                    op=OP.subtract)
            nc.vector.tensor_tensor(out=dwh[:], in0=dwh[:], in1=dwh[:],
                                    op=OP.mult)
            msewh = tt([S, 1], "msewh")
            nc.vector.tensor_reduce(msewh[:], dwh[:], AX.X, OP.add)
            nc.vector.tensor_tensor(out=stack[:, 6:7], in0=mpw[:],
                                    in1=msewh[:], op=OP.mult)

            # ---------------- cross-partition reduce + out ----------------
            red = ps.tile([1, 8], F32, name="red")
            nc.tensor.matmul(out=red[:], lhsT=ct[:, C_ONES:C_ONES + 1],
                             rhs=stack[:], start=True, stop=True)
            osb = sb.tile([1, 8], F32, name="osb")
            nc.vector.tensor_copy(osb[:], red[:])
            nc.sync.dma_start(out=out[:, :], in_=osb[:])
    _split_multiwaits(nc, k=1)
    return nc


_NC_CACHE = None
LAST_RESULTS = None


def _get_nc():
    global _NC_CACHE
    if _NC_CACHE is None:
        _NC_CACHE = build_bass()
    return _NC_CACHE


def run(pyolos, gboxes_ltrb, labels, trace=False, **spmd_kwargs):
    global LAST_RESULTS
    nc = _get_nc()
    py = np.ascontiguousarray(
        np.asarray(pyolos, np.float32).reshape(B, 425, HW))
    gbx = np.ascontiguousarray(np.asarray(gboxes_ltrb, np.float32))
    lbl = np.asarray(labels).astype(np.float32)
    in_maps = []
    for c in range(NC):
        sl = slice(c * BL, (c + 1) * BL)
        in_maps.append({
            "pyolos": py[sl],
            "consts": _make_consts(gbx[sl].reshape(S, 4),
                                   lbl[sl].reshape(S)),
        })
    res = run_bass_kernel_spmd(nc, in_maps, list(range(NC)), trace=trace,
                               **spmd_kwargs)
    LAST_RESULTS = res
    outs = np.stack([r["out"][0] for r in res.results]).astype(np.float64)
    t = outs.sum(0)
    dense_sq, pos_mse, pos_psq, npos, cls_num, txy_s, twh_s = t[:7]
    loss = (5.0 * pos_mse / B
            + (dense_sq - pos_psq) / B
            + cls_num / max(npos, 1.0)
            + txy_s / B
            + twh_s / B)
    return np.float32(loss)


def kernel(pyolos, gboxes_ltrb, labels):
    return run(pyolos, gboxes_ltrb, labels)



# revision 7
# speedup vs baseline: 1.2329x; 1.2329x over previous
"""YOLO-v2 loss kernel for Trainium2 (8 NeuronCores, data-parallel over batch).

v2: all matching logic (which depends only on gboxes/labels, 8KB of input)
is precomputed on the host into per-slot constants + gather offsets; the
device does every computation that touches pyolos:
  - dense conf term: sum sigmoid(conf)^2 over all 16x5x676 positions
    (3 activation passes, bf16 intermediates),
  - an indirect gather of 86 channels per matched slot (128 slots/core),
  - fused per-slot IoU/decode/loss math on DVE+Pool (scalar_tensor_tensor /
    tensor_tensor_reduce keep the op count ~30),
  - outputs [128, 8] per-core partial terms; host does the all-reduce-mean.
"""

import numpy as np

from concourse import bass, mybir
from concourse.bass_utils import run_bass_kernel_spmd
from concourse.tile import TileContext

F32 = mybir.dt.float32
BF16 = mybir.dt.bfloat16
I32 = mybir.dt.int32
AF = mybir.ActivationFunctionType
OP = mybir.AluOpType
AX = mybir.AxisListType

NC = 8                 # cores
B = 128                # batch
BL = B // NC           # images per core (16)
NGT = 8                # GTs per image
S = BL * NGT           # slots per core (128)
GRID = 26
HW = GRID * GRID       # 676
NANC = 5
IMG = 425 * HW         # elements per image (287300)
PL = 5 * HW            # channel stride in elements (3380)
EPS = 1e-7
ANC = np.array([[0.05, 0.07], [0.12, 0.15], [0.25, 0.30],
                [0.45, 0.50], [0.80, 0.85]], np.float32)

# float consts layout [S, CF]
CF_A4 = 0       # [4] = [1,1,0,0]
CF_CR26 = 4     # [2] colrow / 26
CF_GLT = 6      # [2] gbox lt
CF_GRBN = 8     # [2] -gbox rb
CF_AHALF = 10   # [2] anc[idxm] / 2
CF_AREAA = 12   # [1] anchor area
CF_AGEPS = 13   # [1] gt area + eps
CF_ZM1 = 14     # [2] 1 - txy target
CF_TWHT = 16    # [2] twh target
CF_LW6 = 18     # [6] [lastw x4, lastw*weff x2]
CF_N = 24

NCHG = 86       # gathered channels: conf, cls0..79, tx, ty, tw, th, cls_lbl


def _host_match(gbx: np.ndarray, lbl: np.ndarray):
    """Matching for one core's S slots. gbx [S,4] f32 ltrb, lbl [S] int.
    Mirrors reference.match_one in float32. Returns (cf [S,CF_N] f32,
    offs [S,NCHG] i32)."""
    gbx = gbx.astype(np.float32)
    cxy = (gbx[:, :2] + gbx[:, 2:]) * np.float32(0.5)
    wh = gbx[:, 2:] - gbx[:, :2]
    inter = np.minimum(wh[:, None, :], ANC[None]).prod(-1)
    areag = wh.prod(-1)
    areaa5 = (ANC[:, 0] * ANC[:, 1])
    iou2 = inter / (areag[:, None] + areaa5[None] - inter + np.float32(EPS))
    mign = iou2 > 0.5
    idxm = iou2.argmax(-1)
    colrow = (cxy * np.float32(GRID)).astype(np.int32)
    txy = (cxy - colrow.astype(np.float32) / np.float32(GRID)) * np.float32(GRID)
    twh = np.log(wh / ANC[idxm])
    weight = np.float32(2.0) - areag
    cell = colrow[:, 1] * GRID + colrow[:, 0]
    key = cell * NANC + idxm

    lastw = np.ones(S, np.float32)
    ign = np.zeros(S, np.float32)
    for i in range(BL):
        for a in range(NGT):
            s = i * NGT + a
            for j in range(a + 1, NGT):
                t = i * NGT + j
                if key[t] == key[s]:
                    lastw[s] = 0.0
                if cell[t] == cell[s] and mign[t, idxm[s]]:
                    ign[s] = 1.0
    weff = np.where(ign > 0, np.float32(-1.0), weight)

    cf = np.zeros((S, CF_N), np.float32)
    cf[:, CF_A4:CF_A4 + 4] = [1.0, 1.0, 0.0, 0.0]
    cf[:, CF_CR26:CF_CR26 + 2] = colrow.astype(np.float32) / np.float32(GRID)
    cf[:, CF_GLT:CF_GLT + 2] = gbx[:, 0:2]
    cf[:, CF_GRBN:CF_GRBN + 2] = -gbx[:, 2:4]
    cf[:, CF_AHALF:CF_AHALF + 2] = ANC[idxm] * np.float32(0.5)
    cf[:, CF_AREAA] = areaa5[idxm]
    cf[:, CF_AGEPS] = areag + np.float32(EPS)
    cf[:, CF_ZM1:CF_ZM1 + 2] = np.float32(1.0) - txy
    cf[:, CF_TWHT:CF_TWHT + 2] = twh
    cf[:, CF_LW6:CF_LW6 + 4] = lastw[:, None]
    cf[:, CF_LW6 + 4:CF_LW6 + 6] = (lastw * weff)[:, None]

    img = np.arange(S) // NGT
    rowoff = img * IMG + idxm * HW + cell          # element offset of c=0
    offs = np.empty((S, NCHG), np.int64)
    offs[:, 0:85] = rowoff[:, None] + np.arange(85)[None, :] * PL
    offs[:, 85] = rowoff + lbl.astype(np.int64) * PL   # cls channel c=lbl
    return cf, offs.astype(np.int32)


def _split_multiwaits(nc: bass.Bass, k: int = 1) -> None:
    """This walrus build rejects instructions with >~2 sync waits; hoist
    extra waits onto preceding same-engine NoOps (equivalent for monotone
    sem-ge waits)."""
    for fn in nc.m.functions:
        for bb in fn.blocks:
            out = []
            for inst in bb.instructions:
                si = inst.sync_info
                waits = list(si.on_wait) if si is not None and si.on_wait else []
                if len(waits) > k:
                    for i, w in enumerate(waits[:-k]):
                        out.append(mybir.InstNoOp(
                            name=f"{inst.name}-wsplit{i}",
                            engine=inst.engine,
                            bass_nofuse=True,
                            sync_info=mybir.SyncInfo(on_wait=[w],
                                                     on_update=[]),
                        ))
                    inst.sync_info = mybir.SyncInfo(
                        on_wait=waits[-k:], on_update=list(si.on_update))
                out.append(inst)
            bb.instructions = out


def build_bass() -> bass.Bass:
    nc = bass.Bass()
    py = nc.declare_dram_parameter("pyolos", [BL, 425, HW], F32, isOutput=False)
    cfp = nc.declare_dram_parameter("cf", [S, CF_N], F32, isOutput=False)
    oip = nc.declare_dram_parameter("oi", [S, NCHG], I32, isOutput=False)
    outp = nc.declare_dram_parameter("out", [S, 8], F32, isOutput=True)
    py_flat = py[:, :, :].rearrange("a b c -> (a b c)")

    with TileContext(nc) as tc:
        with tc.tile_pool(name="sb", bufs=1) as sb:
            # ---- tiles ----
            conf_t = sb.tile([BL * 5, HW], F32, name="conf_t")
            e80 = sb.tile([BL * 5, HW], BF16, name="e80")
            pf = sb.tile([S, NCHG], F32, name="pf")
            ct = sb.tile([S, CF_N], F32, name="ct")
            oi_t = sb.tile([S, NCHG], I32, name="oi_t")
            stack = sb.tile([S, 8], F32, name="stack")
            q = sb.tile([S, 6], F32, name="q")
            g2 = sb.tile([S, 2], F32, name="g2")

            def tt(shape, tag, dt=F32):
                return sb.tile(shape, dt, name=tag)

            # ---- DMA issues (independent, spread across engines) ----
            nc.sync.dma_start(out=conf_t[:], in_=py[:, 0:5, :])
            nc.scalar.dma_start(out=ct[:], in_=cfp[:, :])
            nc.gpsimd.dma_start(out=oi_t[:], in_=oip[:, :])

            # ---- Pool: memsets (no deps), then gather ----
            nc.gpsimd.memset(stack[:], 0.0)
            nc.gpsimd.memset(q[:], 1.0)
            nc.gpsimd.memset(g2[:], 0.0)
            nc.gpsimd.indirect_dma_start(
                out=pf[:], out_offset=None,
                in_=py_flat.rearrange("(a b) -> a b", b=1),
                in_offset=bass.IndirectOffsetOnAxis(ap=oi_t[:, :], axis=0))

            # ---- Scalar: dense conf chain + slot activations ----
            # sigma(x)^2 = exp(-2*softplus(-x)); Exp/Ln only so one table set.
            nc.scalar.activation(e80[:], conf_t[:], AF.Exp, scale=-1.0)
            e4 = tt([S, 4], "e4")
            nc.scalar.activation(e4[:], pf[:, 81:85], AF.Exp, scale=-1.0)
            e1 = tt([S, 1], "e1")
            nc.scalar.activation(e1[:], pf[:, 0:1], AF.Exp, scale=-1.0)
            spn = tt([S, 2], "spn")
            nc.scalar.activation(spn[:], e4[:, 0:2], AF.Ln, bias=1.0)
            nc.scalar.activation(e80[:], e80[:], AF.Ln, bias=1.0)
            ecls = tt([S, 80], "ecls")
            nc.scalar.activation(ecls[:], pf[:, 1:81], AF.Exp)
            spsum = tt([S, 1], "spsum")
            nc.scalar.activation(ecls[:], ecls[:], AF.Ln, bias=1.0,
                                 accum_out=spsum[:])
            nc.scalar.activation(e80[:], e80[:], AF.Exp, scale=-2.0,
                                 accum_out=stack[0:BL * 5, 0:1])

            # ---- Pool: off-chain slot terms ----
            c4 = tt([S, 4], "c4")
            nc.gpsimd.tensor_tensor(out=c4[:], in0=e4[:],
                                    in1=ct[:, CF_A4:CF_A4 + 4], op=OP.add)
            ce1 = tt([S, 1], "ce1")
            nc.gpsimd.tensor_scalar(ce1[:], e1[:], 1.0, None, OP.add)
            bb = tt([S, 2], "bb")
            nc.gpsimd.tensor_tensor(out=bb[:], in0=pf[:, 81:83],
                                    in1=ct[:, CF_ZM1:CF_ZM1 + 2], op=OP.mult)
            bb2 = tt([S, 2], "bb2")
            nc.gpsimd.tensor_tensor(out=bb2[:], in0=bb[:], in1=spn[:],
                                    op=OP.add)
            nc.gpsimd.tensor_tensor(out=q[:, 4:5], in0=bb2[:, 0:1],
                                    in1=bb2[:, 1:2], op=OP.add)
            dwh = tt([S, 2], "dwh")
            nc.gpsimd.tensor_tensor(out=dwh[:], in0=pf[:, 83:85],
                                    in1=ct[:, CF_TWHT:CF_TWHT + 2],
                                    op=OP.subtract)
            dw2 = tt([S, 2], "dw2")
            nc.gpsimd.tensor_tensor(out=dw2[:], in0=dwh[:], in1=dwh[:],
                                    op=OP.mult)
            nc.gpsimd.tensor_tensor(out=q[:, 5:6], in0=dw2[:, 0:1],
                                    in1=dw2[:, 1:2], op=OP.add)
            nc.gpsimd.tensor_tensor(out=q[:, 3:4], in0=spsum[:],
                                    in1=pf[:, 85:86], op=OP.subtract)

            # ---- DVE: serial IoU/decode chain ----
            r4 = tt([S, 4], "r4")
            nc.vector.reciprocal(r4[:], c4[:])   # [pconf-,sx,sy->?]: [sx? no]
            # r4 = [1/(1+e^-tx), 1/(1+e^-ty), e^tw, e^th]
            pwh2 = tt([S, 2], "pwh2")
            nc.vector.tensor_tensor(out=pwh2[:], in0=r4[:, 2:4],
                                    in1=ct[:, CF_AHALF:CF_AHALF + 2],
                                    op=OP.mult)
            tlt = tt([S, 2], "tlt")
            nc.vector.scalar_tensor_tensor(
                out=tlt[:], in0=r4[:, 0:2], scalar=1.0 / GRID, in1=pwh2[:],
                op0=OP.mult, op1=OP.subtract)
            trb = tt([S, 2], "trb")
            nc.vector.scalar_tensor_tensor(
                out=trb[:], in0=r4[:, 0:2], scalar=-1.0 / GRID, in1=pwh2[:],
                op0=OP.mult, op1=OP.subtract)
            plt = tt([S, 2], "plt")
            nc.vector.tensor_tensor(out=plt[:], in0=tlt[:],
                                    in1=ct[:, CF_CR26:CF_CR26 + 2], op=OP.add)
            prbn = tt([S, 2], "prbn")
            nc.vector.tensor_tensor(out=prbn[:], in0=trb[:],
                                    in1=ct[:, CF_CR26:CF_CR26 + 2],
                                    op=OP.subtract)
            imlt = tt([S, 2], "imlt")
            nc.vector.tensor_tensor(out=imlt[:], in0=plt[:],
                                    in1=ct[:, CF_GLT:CF_GLT + 2], op=OP.max)
            imrbn = tt([S, 2], "imrbn")
            nc.vector.tensor_tensor(out=imrbn[:], in0=prbn[:],
                                    in1=ct[:, CF_GRBN:CF_GRBN + 2], op=OP.max)
            niwh = tt([S, 2], "niwh")
            nc.vector.tensor_tensor(out=niwh[:], in0=imlt[:], in1=imrbn[:],
                                    op=OP.add)
            nc.vector.tensor_scalar(niwh[:], niwh[:], 0.0, None, OP.min)
            inter = tt([S, 1], "inter")
            nc.vector.tensor_tensor(out=inter[:], in0=niwh[:, 0:1],
                                    in1=niwh[:, 1:2], op=OP.mult)
            # pool computes ea while DVE runs the chain
            ea = tt([S, 1], "ea")
            nc.gpsimd.tensor_tensor(out=ea[:], in0=r4[:, 2:3],
                                    in1=r4[:, 3:4], op=OP.mult)
            den = tt([S, 1], "den")
            nc.vector.scalar_tensor_tensor(
                out=den[:], in0=ea[:], scalar=ct[:, CF_AREAA:CF_AREAA + 1],
                in1=inter[:], op0=OP.mult, op1=OP.subtract)
            nc.vector.tensor_scalar(den[:], den[:],
                                    ct[:, CF_AGEPS:CF_AGEPS + 1], None,
                                    OP.add)
            deni = tt([S, 1], "deni")
            nc.vector.reciprocal(deni[:], den[:])
            nc.vector.tensor_tensor(out=g2[:, 0:1], in0=inter[:],
                                    in1=deni[:], op=OP.mult)

            # Pool: masks from gconf
            gpos = tt([S, 1], "gpos")
            nc.gpsimd.tensor_scalar(gpos[:], g2[:, 0:1], 0.0, None, OP.is_gt)
            m6 = tt([S, 6], "m6")
            nc.gpsimd.tensor_scalar(m6[:], ct[:, CF_LW6:CF_LW6 + 6],
                                    gpos[:, 0:1], None, OP.mult)

            # DVE tail: conf terms + final mask multiply
            pc = tt([S, 1], "pc")
            nc.vector.reciprocal(pc[:], ce1[:])
            d2 = tt([S, 2], "d2")
            nc.vector.tensor_tensor(out=d2[:],
                                    in0=pc[:, 0:1].to_broadcast([S, 2]),
                                    in1=g2[:], op=OP.subtract)
            nc.vector.tensor_tensor(out=q[:, 0:2], in0=d2[:], in1=d2[:],
                                    op=OP.mult)
            nc.vector.tensor_tensor(out=stack[:, 1:7], in0=q[:], in1=m6[:],
                                    op=OP.mult)

            # ---- output ----
            nc.sync.dma_start(out=outp[:, :], in_=stack[:])
    _split_multiwaits(nc, k=1)
    return nc


_NC_CACHE = None
LAST_RESULTS = None


def _get_nc():
    global _NC_CACHE
    if _NC_CACHE is None:
        _NC_CACHE = build_bass()
    return _NC_CACHE


def run(pyolos, gboxes_ltrb, labels, trace=False, **spmd_kwargs):
    global LAST_RESULTS
    nc = _get_nc()
    py = np.ascontiguousarray(
        np.asarray(pyolos, np.float32).reshape(B, 425, HW))
    gbx = np.asarray(gboxes_ltrb, np.float32)
    lbl = np.asarray(labels)
    in_maps = []
    for c in range(NC):
        sl = slice(c * BL, (c + 1) * BL)
        cf, offs = _host_match(gbx[sl].reshape(S, 4), lbl[sl].reshape(S))
        in_maps.append({"pyolos": py[sl], "cf": cf, "oi": offs})
    res = run_bass_kernel_spmd(nc, in_maps, list(range(NC)), trace=trace,
                               **spmd_kwargs)
    LAST_RESULTS = res
    t = np.zeros(8, np.float64)
    for r in res.results:
        t += r["out"].astype(np.float64).sum(0)
    dense_sq, pos_mse, pos_psq, npos, cls_num, txy_s, twh_s = t[:7]
    loss = (5.0 * pos_mse / B
            + (dense_sq - pos_psq) / B
            + cls_num / max(npos, 1.0)
            + txy_s / B
            + twh_s / B)
    return np.float32(loss)


def kernel(pyolos, gboxes_ltrb, labels):
    return run(pyolos, gboxes_ltrb, labels)


# revision 9
# speedup vs baseline: 1.3343x; 1.0822x over previous
"""YOLO-v2 loss kernel for Trainium2 (8 NeuronCores, data-parallel over batch).

All matching logic (which depends only on gboxes/labels, 8KB of input) is
precomputed on the host into per-slot constants + gather offsets; the device
does every computation that touches pyolos:
  - dense conf term: sum sigmoid(conf)^2 over all 16x5x676 positions
    (3 activation passes with a per-row accumulate),
  - one indirect gather of 90 channels per matched slot (tx/ty/tw/th are
    gathered twice so lt and rb box corners compute in single [S,4] ops),
  - a fused IoU/decode chain on DVE, off-chain terms on Pool,
  - outputs [128, 7] per-core partial terms; host does the all-reduce-mean.

Gathered column layout: [tx, ty, tx, ty, tw, th, tw, th, conf, cls_lbl,
cls0..cls79].  exp(-x) of cols 0:9 + reciprocal of (e + A9) gives
[sx, sy, sx, sy, e^tw, e^th, e^tw, e^th, pconf] in two ops.
"""

import numpy as np

from concourse import bass, mybir
from concourse.bass_utils import run_bass_kernel_spmd
from concourse.tile import TileContext

F32 = mybir.dt.float32
I32 = mybir.dt.int32
AF = mybir.ActivationFunctionType
OP = mybir.AluOpType
AX = mybir.AxisListType

NC = 8                 # cores
B = 128                # batch
BL = B // NC           # images per core (16)
NGT = 8                # GTs per image
S = BL * NGT           # slots per core (128)
GRID = 26
HW = GRID * GRID       # 676
NANC = 5
IMG = 425 * HW         # elements per image (287300)
PL = 5 * HW            # channel stride in elements (3380)
EPS = 1e-7
ANC = np.array([[0.05, 0.07], [0.12, 0.15], [0.25, 0.30],
                [0.45, 0.50], [0.80, 0.85]], np.float32)

# float consts layout [S, CF_N]
CF_A9 = 0       # [9] = [1,1,1,1,0,0,0,0,1]
CF_SC4 = 9      # [4] = [1/26, 1/26, -1/26, -1/26]
CF_ANCQ = 13    # [4] = [-aw/2, -ah/2, -aw/2, -ah/2]
CF_CRQ = 17     # [4] = [crx, cry, -crx, -cry] / 26
CF_GQ = 21      # [4] = [glx, gly, -grx, -gry]
CF_AREAA = 25   # [1] anchor area
CF_AGEPS = 26   # [1] gt area + eps
CF_ZM1 = 27     # [2] 1 - txy target
CF_TWHT = 29    # [2] twh target
CF_LW6 = 31     # [6] [lastw x4, lastw*weff x2]
CF_N = 37

NCHG = 90       # gathered channels


def _host_match(gbx: np.ndarray, lbl: np.ndarray):
    """Matching for one core's S slots. gbx [S,4] f32 ltrb, lbl [S] int.
    Mirrors reference.match_one in float32. Returns (cf [S,CF_N] f32,
    offs [S,NCHG] i32)."""
    gbx = gbx.astype(np.float32)
    cxy = (gbx[:, :2] + gbx[:, 2:]) * np.float32(0.5)
    wh = gbx[:, 2:] - gbx[:, :2]
    inter = np.minimum(wh[:, None, :], ANC[None]).prod(-1)
    areag = wh.prod(-1)
    areaa5 = (ANC[:, 0] * ANC[:, 1])
    iou2 = inter / (areag[:, None] + areaa5[None] - inter + np.float32(EPS))
    mign = iou2 > 0.5
    idxm = iou2.argmax(-1)
    colrow = (cxy * np.float32(GRID)).astype(np.int32)
    txy = (cxy - colrow.astype(np.float32) / np.float32(GRID)) * np.float32(GRID)
    twh = np.log(wh / ANC[idxm])
    weight = np.float32(2.0) - areag
    cell = colrow[:, 1] * GRID + colrow[:, 0]
    key = cell * NANC + idxm

    lastw = np.ones(S, np.float32)
    ign = np.zeros(S, np.float32)
    for i in range(BL):
        for a in range(NGT):
            s = i * NGT + a
            for j in range(a + 1, NGT):
                t = i * NGT + j
                if key[t] == key[s]:
                    lastw[s] = 0.0
                if cell[t] == cell[s] and mign[t, idxm[s]]:
                    ign[s] = 1.0
    weff = np.where(ign > 0, np.float32(-1.0), weight)

    cr26 = colrow.astype(np.float32) / np.float32(GRID)
    cf = np.zeros((S, CF_N), np.float32)
    cf[:, CF_A9:CF_A9 + 9] = [1, 1, 1, 1, 0, 0, 0, 0, 1]
    cf[:, CF_SC4:CF_SC4 + 4] = np.float32(1.0) / GRID * np.array([1, 1, -1, -1])
    ah = ANC[idxm] * np.float32(-0.5)
    cf[:, CF_ANCQ:CF_ANCQ + 4] = np.concatenate([ah, ah], 1)
    cf[:, CF_CRQ:CF_CRQ + 4] = np.concatenate([cr26, -cr26], 1)
    cf[:, CF_GQ:CF_GQ + 4] = np.concatenate([gbx[:, 0:2], -gbx[:, 2:4]], 1)
    cf[:, CF_AREAA] = areaa5[idxm]
    cf[:, CF_AGEPS] = areag + np.float32(EPS)
    cf[:, CF_ZM1:CF_ZM1 + 2] = np.float32(1.0) - txy
    cf[:, CF_TWHT:CF_TWHT + 2] = twh
    cf[:, CF_LW6:CF_LW6 + 4] = lastw[:, None]
    cf[:, CF_LW6 + 4:CF_LW6 + 6] = (lastw * weff)[:, None]

    img = np.arange(S) // NGT
    rowoff = img * IMG + idxm * HW + cell          # element offset of c=0
    chan = np.empty((S, NCHG), np.int64)
    chan[:, 0:8] = np.array([81, 82, 81, 82, 83, 84, 83, 84])[None, :]
    chan[:, 8] = 0
    chan[:, 9] = lbl.astype(np.int64)              # cls channel c = lbl
    chan[:, 10:90] = np.arange(1, 81)[None, :]
    offs = rowoff[:, None] + chan * PL
    return cf, offs.astype(np.int32)


def _split_multiwaits(nc: bass.Bass, k: int = 1) -> None:
    """This walrus build rejects instructions with >~2 sync waits; hoist
    extra waits onto preceding same-engine NoOps (equivalent for monotone
    sem-ge waits)."""
    for fn in nc.m.functions:
        for bb in fn.blocks:
            out = []
            for inst in bb.instructions:
                si = inst.sync_info
                waits = list(si.on_wait) if si is not None and si.on_wait else []
                if len(waits) > k:
                    for i, w in enumerate(waits[:-k]):
                        out.append(mybir.InstNoOp(
                            name=f"{inst.name}-wsplit{i}",
                            engine=inst.engine,
                            bass_nofuse=True,
                            sync_info=mybir.SyncInfo(on_wait=[w],
                                                     on_update=[]),
                        ))
                    inst.sync_info = mybir.SyncInfo(
                        on_wait=waits[-k:], on_update=list(si.on_update))
                out.append(inst)
            bb.instructions = out


def build_bass() -> bass.Bass:
    nc = bass.Bass()
    py = nc.declare_dram_parameter("pyolos", [BL, 425, HW], F32, isOutput=False)
    cfp = nc.declare_dram_parameter("cf", [S, CF_N], F32, isOutput=False)
    oip = nc.declare_dram_parameter("oi", [S, NCHG], I32, isOutput=False)
    outp = nc.declare_dram_parameter("out", [S, 7], F32, isOutput=True)
    py_flat = py[:, :, :].rearrange("a b c -> (a b c)")

    with TileContext(nc) as tc:
        with tc.tile_pool(name="sb", bufs=1) as sb:
            # ---- tiles ----
            conf_t = sb.tile([BL * 5, HW], F32, name="conf_t")
            e80 = sb.tile([BL * 5, HW], F32, name="e80")
            pf = sb.tile([S, NCHG], F32, name="pf")
            ct = sb.tile([S, CF_N], F32, name="ct")
            oi_t = sb.tile([S, NCHG], I32, name="oi_t")
            stack = sb.tile([S, 7], F32, name="stack")
            q = sb.tile([S, 6], F32, name="q")
            g2 = sb.tile([S, 2], F32, name="g2")

            def tt(shape, tag, dt=F32):
                return sb.tile(shape, dt, name=tag)

            # ---- DMA issues: conf first (feeds the long scalar act chain),
            # oi on the scalar HW queue in parallel, cf second on sync ----
            nc.sync.dma_start(out=conf_t[:], in_=py[:, 0:5, :])
            nc.scalar.dma_start(out=oi_t[:], in_=oip[:, :])
            nc.sync.dma_start(out=ct[:], in_=cfp[:, :])

            # ---- Pool: memsets (no deps), then gather ----
            nc.gpsimd.memset(stack[64:S, 0:1], 0.0)  # rows 64:80 overwritten
            # by the dense accum below; partition starts must be mult. of 32
            nc.gpsimd.memset(q[:], 1.0)
            nc.gpsimd.memset(g2[:], 0.0)
            nc.gpsimd.indirect_dma_start(
                out=pf[:], out_offset=None,
                in_=py_flat.rearrange("(a b) -> a b", b=1),
                in_offset=bass.IndirectOffsetOnAxis(ap=oi_t[:, :], axis=0))

            # ---- Scalar: dense conf chain + slot activations ----
            # sigma(x)^2 = exp(-2*softplus(-x)); Exp/Ln only -> one table set.
            nc.scalar.activation(e80[:], conf_t[:], AF.Exp, scale=-1.0)
            nc.scalar.activation(e80[:], e80[:], AF.Ln, bias=1.0)
            e9 = tt([S, 9], "e9")
            nc.scalar.activation(e9[:], pf[:, 0:9], AF.Exp, scale=-1.0)
            spn = tt([S, 2], "spn")
            nc.scalar.activation(spn[:], e9[:, 0:2], AF.Ln, bias=1.0)
            ecls = tt([S, 80], "ecls")
            nc.scalar.activation(ecls[:], pf[:, 10:90], AF.Exp)
            spsum = tt([S, 1], "spsum")
            nc.scalar.activation(ecls[:], ecls[:], AF.Ln, bias=1.0,
                                 accum_out=spsum[:])
            nc.scalar.activation(e80[:], e80[:], AF.Exp, scale=-2.0,
                                 accum_out=stack[0:BL * 5, 0:1])

            # ---- Pool: off-chain slot terms ----
            c9 = tt([S, 9], "c9")
            nc.gpsimd.tensor_tensor(out=c9[:], in0=e9[:],
                                    in1=ct[:, CF_A9:CF_A9 + 9], op=OP.add)
            bb = tt([S, 2], "bb")
            nc.gpsimd.tensor_tensor(out=bb[:], in0=pf[:, 0:2],
                                    in1=ct[:, CF_ZM1:CF_ZM1 + 2], op=OP.mult)
            bb2 = tt([S, 2], "bb2")
            nc.gpsimd.tensor_tensor(out=bb2[:], in0=bb[:], in1=spn[:],
                                    op=OP.add)
            nc.gpsimd.tensor_tensor(out=q[:, 4:5], in0=bb2[:, 0:1],
                                    in1=bb2[:, 1:2], op=OP.add)
            dwh = tt([S, 2], "dwh")
            nc.gpsimd.tensor_tensor(out=dwh[:], in0=pf[:, 4:6],
                                    in1=ct[:, CF_TWHT:CF_TWHT + 2],
                                    op=OP.subtract)
            dw2 = tt([S, 2], "dw2")
            nc.gpsimd.tensor_tensor(out=dw2[:], in0=dwh[:], in1=dwh[:],
                                    op=OP.mult)
            nc.gpsimd.tensor_tensor(out=q[:, 5:6], in0=dw2[:, 0:1],
                                    in1=dw2[:, 1:2], op=OP.add)
            nc.gpsimd.tensor_tensor(out=q[:, 3:4], in0=spsum[:],
                                    in1=pf[:, 9:10], op=OP.subtract)

            # ---- DVE: IoU/decode chain ----
            r9 = tt([S, 9], "r9")
            nc.vector.reciprocal(r9[:], c9[:])
            # r9 = [sx, sy, sx, sy, e^tw, e^th, e^tw, e^th, pconf]
            # pool computes the pred-area denominator pieces off-chain
            ea = tt([S, 1], "ea")
            nc.gpsimd.tensor_tensor(out=ea[:], in0=r9[:, 4:5],
                                    in1=r9[:, 5:6], op=OP.mult)
            eag = tt([S, 1], "eag")
            nc.gpsimd.tensor_scalar(eag[:], ea[:],
                                    ct[:, CF_AREAA:CF_AREAA + 1],
                                    ct[:, CF_AGEPS:CF_AGEPS + 1],
                                    OP.mult, OP.add)
            # DVE chain continues
            v4 = tt([S, 4], "v4")
            nc.vector.tensor_tensor(out=v4[:], in0=r9[:, 0:4],
                                    in1=ct[:, CF_SC4:CF_SC4 + 4], op=OP.mult)
            u4 = tt([S, 4], "u4")
            nc.vector.tensor_tensor(out=u4[:], in0=r9[:, 4:8],
                                    in1=ct[:, CF_ANCQ:CF_ANCQ + 4],
                                    op=OP.mult)
            w4 = tt([S, 4], "w4")
            nc.vector.tensor_tensor(out=w4[:], in0=v4[:], in1=u4[:],
                                    op=OP.add)
            pltn = tt([S, 4], "pltn")
            nc.vector.tensor_tensor(out=pltn[:], in0=w4[:],
                                    in1=ct[:, CF_CRQ:CF_CRQ + 4], op=OP.add)
            im4 = tt([S, 4], "im4")
            nc.vector.tensor_tensor(out=im4[:], in0=pltn[:],
                                    in1=ct[:, CF_GQ:CF_GQ + 4], op=OP.max)
            niwh = tt([S, 2], "niwh")
            nc.vector.tensor_tensor(out=niwh[:], in0=im4[:, 0:2],
                                    in1=im4[:, 2:4], op=OP.add)
            nc.vector.tensor_scalar(niwh[:], niwh[:], 0.0, None, OP.min)
            inter = tt([S, 1], "inter")
            nc.vector.tensor_tensor(out=inter[:], in0=niwh[:, 0:1],
                                    in1=niwh[:, 1:2], op=OP.mult)
            den = tt([S, 1], "den")
            nc.vector.tensor_tensor(out=den[:], in0=eag[:], in1=inter[:],
                                    op=OP.subtract)
            deni = tt([S, 1], "deni")
            nc.vector.reciprocal(deni[:], den[:])
            nc.vector.tensor_tensor(out=g2[:, 0:1], in0=inter[:],
                                    in1=deni[:], op=OP.mult)
            gpos = tt([S, 1], "gpos")
            nc.vector.tensor_scalar(gpos[:], g2[:, 0:1], 0.0, None, OP.is_gt)
            m6 = tt([S, 6], "m6")
            nc.vector.tensor_scalar(m6[:], ct[:, CF_LW6:CF_LW6 + 6],
                                    gpos[:, 0:1], None, OP.mult)
            d2 = tt([S, 2], "d2")
            nc.vector.tensor_tensor(out=d2[:],
                                    in0=r9[:, 8:9].to_broadcast([S, 2]),
                                    in1=g2[:], op=OP.subtract)
            nc.vector.tensor_tensor(out=q[:, 0:2], in0=d2[:], in1=d2[:],
                                    op=OP.mult)
            nc.vector.tensor_tensor(out=stack[:, 1:7], in0=q[:], in1=m6[:],
                                    op=OP.mult)

            # ---- output ----
            nc.sync.dma_start(out=outp[:, :], in_=stack[:])
    _split_multiwaits(nc, k=1)
    return nc


_NC_CACHE = None
LAST_RESULTS = None


def _get_nc():
    global _NC_CACHE
    if _NC_CACHE is None:
        _NC_CACHE = build_bass()
    return _NC_CACHE


def run(pyolos, gboxes_ltrb, labels, trace=False, **spmd_kwargs):
    global LAST_RESULTS
    nc = _get_nc()
    py = np.ascontiguousarray(
        np.asarray(pyolos, np.float32).reshape(B, 425, HW))
    gbx = np.asarray(gboxes_ltrb, np.float32)
    lbl = np.asarray(labels)
    in_maps = []
    for c in range(NC):
        sl = slice(c * BL, (c + 1) * BL)
        cf, offs = _host_match(gbx[sl].reshape(S, 4), lbl[sl].reshape(S))
        in_maps.append({"pyolos": py[sl], "cf": cf, "oi": offs})
    res = run_bass_kernel_spmd(nc, in_maps, list(range(NC)), trace=trace,
                               **spmd_kwargs)
    LAST_RESULTS = res
    t = np.zeros(7, np.float64)
    for r in res.results:
        t += r["out"].astype(np.float64).sum(0)
    dense_sq, pos_mse, pos_psq, npos, cls_num, txy_s, twh_s = t
    loss = (5.0 * pos_mse / B
            + (dense_sq - pos_psq) / B
            + cls_num / max(npos, 1.0)
            + txy_s / B
            + twh_s / B)
    return np.float32(loss)


def kernel(pyolos, gboxes_ltrb, labels):
    return run(pyolos, gboxes_ltrb, labels)
